# revision 59
# baseline (speedup 1.0000x reference)
"""GroupedQueryAttention Trainium2 Bass kernel (8 NeuronCores, SPMD).

Reference quirk exploited: K/V are tiled R=4x along the group axis and
attention runs over the full concatenated 2048-dim. Mathematically this
collapses:
  scores = Q . tile(K)  ==  (sum of Q's four 512-chunks) . K      (512-dim)
  Z      = attn . tile(V)  -> tiled copies of  attn . V           (512-dim)
  out    = Z @ proj     ==  (attn . V) @ (sum of proj's 4 row-blocks)
So the whole module reduces to a single 512-dim attention:
  Qc = x @ WQc.T + bQc   (WQc = sum of WQ row-blocks)
  K  = x @ WK.T + bK ; V = x @ WV.T + bV
  S  = Qc K^T (causal), softmax, /sqrt(128)
  y  = (softmax(S)/sqrt(128) V) @ projc    (projc = sum of proj row-blocks)

Sharding: 8 cores = 4 batches x 2 (interleaved 128-row blocks). Core with
pairpos q of batch b owns rows {256g+128q .. +127, g=0..7}. Causal key
extents round up to 256-multiples, giving every core the identical
(tile-count, last-tile-width) sequence [(1,256),(1,512),...,(4,512)]
-> a single SPMD program. Per-core masks are host-prepared inputs.

All matmul operands are fp16 (full PE rate, 1 column/cycle; fp32 PSUM
accumulation; tf32-class 10-bit mantissa keeps the exp-sensitive S logits
accurate where bf16 fails tolerance); x^T resident in SBUF (loaded once,
the rt=0 pieces packed with wk|wv into one DMA stream); Z^T produced
directly with V as the stationary operand (no Z transposes; softmax
normalization folds into the output-projection copy via per-partition ACT
scale); E^T transposes batched per key tile into one PSUM bank with a
single DVE copy; blocks processed in descending-size order with a
two-block S lookahead and the previous block's O-projection dtiles woven
between the current block's Z key tiles; S staging and output stores in
fp16; Q's last four chunks run group-major so the eight PSUM drains
overlap trailing matmuls; 1/sqrt(128) folded into proj on the host; DMAs
consolidated because each costs ~565ns issue + ~625ns exclusive HWDGE
occupancy; PSUM drains split between the Scalar and Vector engines.
"""

import numpy as np

import concourse.bacc as bacc
import concourse.mybir as mybir
from concourse.tile import TileContext
from concourse.bass_utils import run_bass_kernel_spmd

B, T, D = 4, 2048, 2048
HD = 512                 # collapsed head dim
NCORES = 8
RB = 8                   # 128-row blocks per core
KT_TILES = [1, 1, 2, 2, 3, 3, 4, 4]   # 512-wide key tiles per block
N_LAST = [256, 512, 256, 512, 256, 512, 256, 512]  # last-tile width
DCH = D // 128           # 16 contraction chunks
dt = mybir.dt
NEG = -1.0e30


def build_kernel():
    nc = bacc.Bacc(None, target_bir_lowering=False)

    xT_d = nc.dram_tensor("xT", [D, T], dt.float16, kind="ExternalInput")
    xTq_d = nc.dram_tensor("xTq", [D, 1024], dt.float16, kind="ExternalInput")
    # per 128-row chunk: [x-piece rt0 (512) | wk (512) | wv (512)] so stage P's
    # opening pass streams one DMA per chunk instead of two
    XW_d = nc.dram_tensor("XW", [D, 512 + 2 * HD], dt.float16, kind="ExternalInput")
    WQT_d = nc.dram_tensor("WQT", [D, HD], dt.float16, kind="ExternalInput")
    PRJ_d = nc.dram_tensor("PRJ", [HD, D], dt.float16, kind="ExternalInput")
    BKQ_d = nc.dram_tensor("BKQ", [128, 8], dt.float32, kind="ExternalInput")
    BVB_d = nc.dram_tensor("BVB", [128, HD], dt.float32, kind="ExternalInput")
    MSK_d = nc.dram_tensor("MSK", [128, 256 * RB], dt.float32, kind="ExternalInput")
    IDT_d = nc.dram_tensor("IDT", [128, 128], dt.float16, kind="ExternalInput")
    OUT_d = nc.dram_tensor("out", [1024, D], dt.float16, kind="ExternalOutput")

    Act = mybir.ActivationFunctionType
    Ax = mybir.AxisListType

    with TileContext(nc) as tc:
        with tc.tile_pool(name="persist", bufs=1) as pp:
            # ---- persistent tiles ------------------------------------------
            # x^T resident as [128,512] pieces so the DMA stream can follow
            # the rt-major compute order (piece (c,rt) arrives just in time);
            # the rt=0 piece lives inside the packed xw tiles instead
            XT_sb = [[None] + [pp.tile([128, 512], dt.float16, tag=f"XT{c}_{rt}",
                                       name=f"XT{c}_{rt}") for rt in range(1, 4)]
                     for c in range(DCH)]
            KT_sb = [pp.tile([128, T], dt.float16, tag=f"KT{h}", name=f"KT{h}")
                     for h in range(4)]
            V_sb = [pp.tile([128, HD], dt.float16, tag=f"V{k}", name=f"V{k}")
                    for k in range(16)]
            # QT split per (h, rt-half) so attention's first blocks only
            # depend on the rt=0 drains
            QT_sb = {(h, rt): pp.tile([128, 512], dt.float16, tag=f"QT{h}_{rt}",
                                      name=f"QT{h}_{rt}")
                     for h in range(4) for rt in range(2)}
            MSK_sb = pp.tile([128, 256 * RB], dt.float32, tag="MSK", name="MSKs")
            ident = pp.tile([128, 128], dt.float16, tag="ident")
            bvb = pp.tile([128, HD], dt.float32, tag="bvb")
            bkq = pp.tile([128, 8], dt.float32, tag="bkq")

            # ---- stage P: projections --------------------------------------
            # x^T loaded once, resident. Per 512-key group: 4 PSUM banks
            # accumulate K^T hd-tiles ([hd, keys], bias added in the ACT
            # drain) and 4 banks accumulate V key-blocks in [keys, hd] layout
            # (x chunk as lhsT); V bias is added by the DVE drain against a
            # host-broadcast bV tile.
            # one PSUM pool spans projections AND attention: attention
            # tiles reuse the kps/vps bank tags, so the cross-phase wait is
            # per-bank (hidden by the Q drain stagger) instead of a
            # whole-pool barrier
            psp_ctx = tc.tile_pool(name="psP", bufs=1, space="PSUM")
            psp = psp_ctx.__enter__()
            with tc.tile_pool(name="wpool", bufs=1) as wp, \
                 tc.tile_pool(name="xstream", bufs=4) as xp:
                # xw[c] = [x-piece rt0 | wk | wv]; weights stay resident
                xw = [wp.tile([128, 512 + 2 * HD], dt.float16, tag=f"xw{c}",
                              name=f"xw{c}") for c in range(DCH)]
                nc.sync.dma_start(out=xw[0][:, 0:640], in_=XW_d[0:128, 0:640])
                nc.sync.dma_start(out=xw[0][:, 640:1536], in_=XW_d[0:128, 640:1536])
                for rt in range(4):                      # key 512-col groups
                    kps = [psp.tile([128, 512], dt.float32, tag=f"kps{h}",
                                    name=f"kps{h}") for h in range(4)]
                    vps = [psp.tile([128, 512], dt.float32, tag=f"vps{j}",
                                    name=f"vps{j}") for j in range(4)]
                    for c in range(DCH):
                        if rt == 0 and c == 3:
                            # constants: not needed until the rt-0 drains, so
                            # keep them out of the startup critical DMA path
                            nc.sync.dma_start(out=ident[:], in_=IDT_d[:])
                            nc.sync.dma_start(out=bvb[:], in_=BVB_d[:])
                            nc.sync.dma_start(out=bkq[:], in_=BKQ_d[:])
                        if rt == 0:   # stream x pieces rt-major + weights
                            if c + 1 < DCH:
                                nc.sync.dma_start(
                                    out=xw[c + 1][:],
                                    in_=XW_d[128 * (c + 1):128 * (c + 1) + 128, :])
                            else:
                                for rt2 in range(1, 4):
                                    for c2 in range(DCH):
                                        nc.sync.dma_start(
                                            out=XT_sb[c2][rt2][:],
                                            in_=xT_d[128 * c2:128 * c2 + 128,
                                                     512 * rt2:512 * rt2 + 512])
                        if c >= DCH - 2:
                            continue   # last two chunks run group-major below
                        xtile = xw[c] if rt == 0 else XT_sb[c][rt]
                        for h in range(4):
                            nc.tensor.matmul(kps[h][:],
                                             xw[c][:, 512 + 128 * h:512 + 128 * h + 128],
                                             xtile[:, 0:512],
                                             start=(c == 0), stop=False)
                        for j in range(4):
                            nc.tensor.matmul(vps[j][:],
                                             xtile[:, 128 * j:128 * j + 128],
                                             xw[c][:, 1024:1536],
                                             start=(c == 0), stop=False)
                    # last two chunks group-major: the 8 accumulation groups
                    # stop staggered, so their drains overlap the trailing
                    # matmuls and the next rt (or Q) never waits on a bank
                    for h in range(4):
                        for c in (DCH - 2, DCH - 1):
                            xtile = xw[c] if rt == 0 else XT_sb[c][rt]
                            nc.tensor.matmul(kps[h][:],
                                             xw[c][:, 512 + 128 * h:512 + 128 * h + 128],
                                             xtile[:, 0:512],
                                             start=False, stop=(c == DCH - 1))
                    for j in range(4):
                        for c in (DCH - 2, DCH - 1):
                            xtile = xw[c] if rt == 0 else XT_sb[c][rt]
                            nc.tensor.matmul(vps[j][:],
                                             xtile[:, 128 * j:128 * j + 128],
                                             xw[c][:, 1024:1536],
                                             start=False, stop=(c == DCH - 1))
                    # drains: K^T bias-adds on ACT, V bias-adds on DVE
                    for h in range(4):
                        nc.scalar.activation(
                            KT_sb[h][:, 512 * rt:512 * rt + 512], kps[h][:],
                            Act.Identity, bias=bkq[:, h:h + 1], scale=1.0)
                    for j in range(4):
                        nc.vector.tensor_add(V_sb[4 * rt + j][:], vps[j][:],
                                             bvb[:])
                # Qc^T for this core's 1024 rows: c-outer, both rt halves in
                # PSUM at once (8 banks), x-query streamed as [128,1024] tiles.
                wq = []
                for c in range(DCH):
                    tq = wp.tile([128, HD], dt.float16, tag=f"wq{c}", name=f"wq{c}")
                    nc.sync.dma_start(out=tq[:], in_=WQT_d[128 * c:128 * c + 128, :])
                    wq.append(tq)
                qps = {}
                for rt in range(2):
                    for h in range(4):
                        # rt=1 on the vps banks: the attention pool reuses
                        # high banks first, and rt=1 drains first
                        tagbank = f"kps{h}" if rt == 1 else f"vps{h}"
                        qps[rt, h] = psp.tile([128, 512], dt.float32,
                                              tag=tagbank, name=f"qps{rt}{h}")
                # chunks 0..11 c-major; the last 4 chunks run group-major so
                # the 8 PSUM accumulation groups stop staggered (~0.85us
                # apart) and their ACT/DVE drains fully overlap the trailing
                # matmuls instead of serializing after Q's last instruction.
                xq_t = []
                for c in range(DCH):
                    xq = xp.tile([128, 1024], dt.float16, tag="xs", name="xq")
                    nc.sync.dma_start(out=xq[:], in_=xTq_d[128 * c:128 * c + 128, :])
                    xq_t.append(xq)
                    if c < DCH - 4:
                        for rt in (1, 0):   # rt=1 groups close first (drained
                            for h in range(4):  # first; attention starts g=7)
                                nc.tensor.matmul(qps[rt, h][:],
                                                 wq[c][:, 128 * h:128 * h + 128],
                                                 xq[:, 512 * rt:512 * rt + 512],
                                                 start=(c == 0), stop=False)
                for rt in (1, 0):
                    for h in range(4):
                        for c in range(DCH - 4, DCH):
                            nc.tensor.matmul(qps[rt, h][:],
                                             wq[c][:, 128 * h:128 * h + 128],
                                             xq_t[c][:, 512 * rt:512 * rt + 512],
                                             start=False, stop=(c == DCH - 1))
                # attention runs blocks in descending order (g=7 first), so
                # drain rt=1 first; bQ add for the DVE half rides on a
                # tensor_scalar add
                for rt in (1, 0):
                    for h in range(4):
                        # odd groups (including the last-stopping h=3, which
                        # gates the attention PSUM pool) drain on the faster
                        # fp16-out ACT path
                        if h % 2 == 1:
                            nc.scalar.activation(
                                QT_sb[h, rt][:], qps[rt, h][:],
                                Act.Identity, bias=bkq[:, 4 + h:5 + h], scale=1.0)
                        else:
                            nc.vector.tensor_scalar_add(
                                QT_sb[h, rt][:], qps[rt, h][:],
                                bkq[:, 4 + h:5 + h])

            # masks prefetch early (cheap, avoids stalling S(0) behind PRJ)
            nc.sync.dma_start(out=MSK_sb[:], in_=MSK_d[:])

            # ---- stages A+O: attention fused with output projection --------
            with tc.tile_pool(name="attn", bufs=1) as ap, \
                 tc.tile_pool(name="oproj", bufs=1) as op:
                psa = psp
                pctr = {"sps": 0, "etp": 0, "ztp": 0, "ops": 0}
                _pmap = {"sps": ("kps", 0), "etp": ("kps", 2),
                         "ztp": ("vps", 0), "ops": ("vps", 2)}

                def pbank(kind):
                    pre, off = _pmap[kind]
                    i = pctr[kind] % 2
                    pctr[kind] += 1
                    return f"{pre}{off + i}"
                # prefetch output-projection weights while attention runs
                prj = []
                for h in range(4):
                    t = op.tile([128, D], dt.float16, tag=f"prj{h}", name=f"prj{h}")
                    nc.sync.dma_start(out=t[:], in_=PRJ_d[128 * h:128 * h + 128, :])
                    prj.append(t)

                state = {}

                def stage_s(g):
                    """S matmuls + mask + per-tile max for row block g."""
                    ntile = KT_TILES[g]
                    nl = N_LAST[g]
                    mpart = ap.tile([128, 4], dt.float32, tag="mpart", bufs=3,
                                    name="mpart")
                    s_tiles = []
                    for kt in range(ntile):
                        w = 512 if kt < ntile - 1 else nl
                        sps = psa.tile([128, 512], dt.float32,
                                       tag=pbank("sps"), name="sps")
                        for h in range(4):
                            nc.tensor.matmul(
                                sps[:, 0:w],
                                QT_sb[h, g // 4][:, 128 * (g % 4):128 * (g % 4) + 128],
                                KT_sb[h][:, 512 * kt:512 * kt + w],
                                start=(h == 0), stop=(h == 3))
                        # fp16 S staging: ~0.05% logit rounding (tf32-class),
                        # halves the ACT-copy / DVE-max / exp-read costs
                        ssb = ap.tile([128, 512], dt.float16, tag="ssb", bufs=9,
                                      name="ssb")
                        if kt == ntile - 1:
                            if w == 512:
                                nc.scalar.copy(ssb[:, 0:256], sps[:, 0:256])
                            nc.vector.tensor_add(ssb[:, w - 256:w],
                                                 sps[:, w - 256:w],
                                                 MSK_sb[:, 256 * g:256 * g + 256])
                        else:
                            nc.scalar.copy(ssb[:, 0:w], sps[:, 0:w])
                        nc.vector.reduce_max(mpart[:, kt:kt + 1], ssb[:, 0:w],
                                             axis=Ax.X)
                        s_tiles.append(ssb)
                    state[g] = (s_tiles, mpart)

                def stage_e(g):
                    """negmax + exp (bf16) + row sums + 1/sum for block g."""
                    ntile = KT_TILES[g]
                    nl = N_LAST[g]
                    s_tiles, mpart = state[g]
                    negm = ap.tile([128, 1], dt.float32, tag="negm", bufs=2,
                                   name="negm")
                    nc.vector.reduce_max(negm[:], mpart[:, 0:ntile], axis=Ax.X,
                                         negate=True)
                    esum = ap.tile([128, 4], dt.float32, tag="esum", bufs=2,
                                   name="esum")
                    e_tiles = []
                    for kt in range(ntile):
                        w = 512 if kt < ntile - 1 else nl
                        esb = ap.tile([128, 512], dt.float16, tag="esb", bufs=9,
                                      name="esb")
                        nc.scalar.activation(
                            esb[:, 0:w], s_tiles[kt][:, 0:w], Act.Exp,
                            bias=negm[:], scale=1.0,
                            accum_out=esum[:, kt:kt + 1])
                        e_tiles.append(esb)
                    stot = ap.tile([128, 1], dt.float32, tag="stot", bufs=2,
                                   name="stot")
                    nc.vector.reduce_sum(stot[:], esum[:, 0:ntile], axis=Ax.X)
                    inv = ap.tile([128, 1], dt.float32, tag="inv", bufs=3,
                                  name="inv")
                    nc.vector.reciprocal(inv[:], stot[:])
                    state[g] = (e_tiles, inv)

                def stage_t(g):
                    """E^T transposes (batched per tile into one PSUM bank),
                    one DVE copy each."""
                    ntile = KT_TILES[g]
                    nl = N_LAST[g]
                    e_tiles, inv = state.pop(g)
                    ets_tiles = []
                    for kt in range(ntile):
                        w = 512 if kt < ntile - 1 else nl
                        nck = w // 128
                        etp = psa.tile([128, 512], dt.float16,
                                       tag=pbank("etp"), name="etp")
                        for kc in range(nck):
                            nc.tensor.transpose(
                                etp[:, 128 * kc:128 * kc + 128],
                                e_tiles[kt][:, 128 * kc:128 * kc + 128],
                                ident[:])
                        ets = ap.tile([128, 512], dt.float16, tag="ets",
                                      bufs=6, name="ets")
                        nc.vector.tensor_copy(ets[:, 0:w], etp[:, 0:w])
                        ets_tiles.append((ets, nck))
                    state[g, "z"] = (ets_tiles, inv)

                def emit_o_dtile(g, dtile):
                    """One output-projection column tile of block g."""
                    zts, inv = state[g, "o"]
                    ops = psa.tile([128, 512], dt.float32,
                                   tag=pbank("ops"), name="ops")
                    for h in range(4):
                        nc.tensor.matmul(
                            ops[:], zts[:, 128 * h:128 * h + 128],
                            prj[h][:, 512 * dtile:512 * dtile + 512],
                            start=(h == 0), stop=(h == 3))
                    osb = op.tile([128, 512], dt.float16, tag="osb", bufs=4,
                                  name="osb")
                    if dtile % 2 == 0:
                        nc.scalar.activation(osb[:], ops[:], Act.Identity,
                                             bias=0.0, scale=inv[:])
                    else:
                        nc.vector.tensor_scalar_mul(osb[:], ops[:], inv[:])
                    nc.sync.dma_start(
                        out=OUT_d[128 * g:128 * g + 128,
                                  512 * dtile:512 * dtile + 512],
                        in_=osb[:])
                    if dtile == 3:
                        state.pop((g, "o"))

                def stage_z(g, gprev):
                    """Z^T accumulation (V as stationary) with the previous
                    block's O-projection dtiles interleaved between key tiles
                    (keeps PE fed while DVE finishes the E^T copies), then
                    Z^T copy to SBUF."""
                    ets_tiles, inv = state.pop((g, "z"))
                    ztp = psa.tile([128, 512], dt.float32,
                                   tag=pbank("ztp"), name="ztp")
                    nkc_tot = sum(nck for _, nck in ets_tiles)
                    od = 0

                    def odrain():
                        nonlocal od
                        if gprev is not None and od < 4:
                            emit_o_dtile(gprev, od)
                            od += 1

                    nmm = 0
                    for kt, (ets, nck) in enumerate(ets_tiles):
                        odrain()
                        for kc in range(nck):
                            kg = 4 * kt + kc
                            for j in range(4):
                                nc.tensor.matmul(
                                    ztp[:, 128 * j:128 * j + 128],
                                    V_sb[kg][:, 128 * j:128 * j + 128],
                                    ets[:, 128 * kc:128 * kc + 128],
                                    start=(nmm == 0), stop=(nmm == 4 * nkc_tot - 1),
                                    skip_group_check=True)
                                nmm += 1
                    zts = ap.tile([128, 512], dt.float16, tag="zts", bufs=3,
                                  name="zts")
                    nc.vector.tensor_copy(zts[:], ztp[:])
                    while gprev is not None and od < 4:
                        emit_o_dtile(gprev, od)
                        od += 1
                    state[g, "o"] = (zts, inv)

                # software pipeline over blocks in DESCENDING size order:
                # big blocks first warm the pipe, tiny blocks end it (short
                # tail). PE order per iteration is
                #   S(next) matmuls, E^T(cur) transposes, then Z(cur) matmuls
                # with O(prev) dtiles woven between Z key tiles.
                order = list(range(RB - 1, -1, -1))
                stage_s(order[0])
                stage_s(order[1])
                stage_e(order[0])
                for i, g in enumerate(order):
                    stage_t(g)
                    if i + 2 < RB:
                        stage_s(order[i + 2])
                    if i + 1 < RB:
                        stage_e(order[i + 1])
                    stage_z(g, order[i - 1] if i >= 1 else None)
                for dtile in range(4):
                    emit_o_dtile(order[-1], dtile)
            psp_ctx.__exit__(None, None, None)

    nc.compile()
    return nc


def host_prep(x, WQ, bQ, WK, bK, WV, bV, proj):
    """Collapse weights, transpose layouts, build per-core input maps."""
    
    f16 = np.float16

    x = np.ascontiguousarray(x, dtype=np.float32)
    WQc = WQ.reshape(4, HD, D).sum(0)
    bQc = bQ.reshape(4, HD).sum(0)
    projc = proj.reshape(4, HD, D).sum(0) / np.sqrt(np.float32(128.0))

    WQT = np.ascontiguousarray(WQc.T).astype(f16)     # [D, HD]
    WKV = np.concatenate([WK.T, WV.T], axis=1)         # [D, 2*HD]
    WKV = np.ascontiguousarray(WKV).astype(f16)
    PRJ = np.ascontiguousarray(projc).astype(f16)     # [HD, D]
    BKQ = np.stack([bK.reshape(4, 128)[h] for h in range(4)] +
                   [bQc.reshape(4, 128)[h] for h in range(4)],
                   axis=1).astype(np.float32)          # [128, 8]
    BVB = np.broadcast_to(bV.reshape(1, HD),
                          (128, HD)).astype(np.float32).copy()
    idt = np.eye(128, dtype=f16)

    in_maps = []
    for core in range(NCORES):
        b, q = divmod(core, 2)
        xT = np.ascontiguousarray(x[b].T)               # [D, T] fp32
        rows = np.concatenate(
            [np.arange(256 * g + 128 * q, 256 * g + 128 * q + 128)
             for g in range(RB)])
        xTq = np.ascontiguousarray(xT[:, rows]).astype(f16)   # [D, 1024]
        xTb = xT.astype(f16)
        XW = np.ascontiguousarray(
            np.concatenate([xTb[:, 0:512], WKV], axis=1))     # [D, 1536]
        msk = np.zeros((128, 256 * RB), dtype=np.float32)
        for g in range(RB):
            ntile = KT_TILES[g]
            nl = N_LAST[g]
            base = 512 * (ntile - 1) + nl - 256   # first masked key col
            key = base + np.arange(256)[None, :]
            row = (256 * g + 128 * q + np.arange(128))[:, None]
            msk[:, 256 * g:256 * g + 256] = np.where(key <= row, 0.0, NEG)
        in_maps.append({
            "xT": xTb, "xTq": xTq, "XW": XW, "WQT": WQT,
            "PRJ": PRJ, "BKQ": BKQ, "BVB": BVB,
            "MSK": msk, "IDT": idt,
        })
    return in_maps


def assemble(results):
    """Gather per-core [1024, D] outputs into [B, T, D]."""
    y = np.empty((B, T, D), dtype=np.float32)
    for core in range(NCORES):
        b, q = divmod(core, 2)
        o = results[core]["out"]
        for g in range(RB):
            y[b, 256 * g + 128 * q:256 * g + 128 * q + 128] = \
                o[128 * g:128 * g + 128]
    return y


_NC_CACHE = None
TRACE = False          # set by test.py to capture a HW profile
LAST_RESULT = None     # BassKernelResults of the most recent run


def kernel(x, WQ, bQ, WK, bK, WV, bV, proj):
    global _NC_CACHE, LAST_RESULT
    in_maps = host_prep(np.asarray(x), np.asarray(WQ), np.asarray(bQ),
                        np.asarray(WK), np.asarray(bK), np.asarray(WV),
                        np.asarray(bV), np.asarray(proj))
    if _NC_CACHE is None:
        _NC_CACHE = build_kernel()
    kw = {}
    if TRACE:
        kw = dict(trace=True, tmpdir="/tmp/bass_trace")
    res = run_bass_kernel_spmd(_NC_CACHE, in_maps, list(range(NCORES)), **kw)
    LAST_RESULT = res
    return assemble(res.results)
